# revision 13
# baseline (speedup 1.0000x reference)
"""Trainium2 Bass kernel for nn_FDModel_18433999634973.

The reference's attention pooling applies softmax over a singleton axis, so
the attention weights are identically 1.0 and each pooled embedding is just a
sum over the K axis.  The model therefore reduces to:

    p?   = sum_k X?[b, k, :]                      (for author/title/text)
    s?   = dot(p?, Wf?[0]) + bf?
    score  = sigmoid([sa, st, sx])                [B, 3]
    logits = score @ Wc.T + bc                    [B, 2]
    out    = softmax(logits, axis=1)

Sharding: pure data parallel over batch (512 -> 8 x 64).  Per core the k-sum
runs on TensorE: a 0/1 selector matrix as the stationary operand contracts
the 128-partition dim (= GB batch rows x KP k-rows), accumulating into PSUM.
The tiny heads run on VectorE/ScalarE.

The embeddings are cast to fp16 on the host: half the HBM traffic (the kernel
is memory-bound) at ~2e-4 extra relative error.  Measured on 8 cores:
~169 us/exec (repeat-delta timing), ~321 GB/s/core sustained = ~90% of the
per-core HBM limit.
"""

import numpy as np

import concourse.bacc as bacc
import concourse.mybir as mybir
import concourse.tile as tile
from concourse.bass_utils import run_bass_kernel_spmd

N_CORES = 8
B = 512
B_SH = B // N_CORES  # 64
KA, KT, KX = 8, 32, 512
DA, DS = 256, 768

# wpack column offsets
OFF_WFX = 0
OFF_WFT = DS
OFF_WFA = 2 * DS
OFF_WC0 = 2 * DS + DA
OFF_WC1 = OFF_WC0 + 3
OFF_B3 = OFF_WC1 + 3
OFF_BC = OFF_B3 + 3
WPACK = OFF_BC + 2  # 1800

F32 = mybir.dt.float32
AL = mybir.AluOpType
ACT = mybir.ActivationFunctionType


def build_module(b_sh: int = B_SH, mm_mode: str = "f16", repeat: int = 1):
    nc = bacc.Bacc(
        "TRN2",
        target_bir_lowering=False,
        debug=False,
        enable_asserts=True,
        num_devices=N_CORES,
    )
    # Stage-1 streaming dtype:
    #  f16  - host casts the embeddings to fp16: half the HBM traffic, PE at
    #         1 cycle/row; final rel err ~2e-4 (fp16 has 10 mantissa bits and
    #         the accumulate stays fp32 in PSUM).
    #  f32r - fp32 bits at 1 cycle/row (PE rounds the operands); ~4.6e-4.
    #  f32  - exact fp32, PE at 4 cycles/row (PE-bound).
    MDT = {"f16": mybir.dt.float16, "f32r": mybir.dt.float32r, "f32": F32}[mm_mode]
    xt = nc.dram_tensor("xt", [b_sh, KX, DS], MDT, kind="ExternalInput")
    xs = nc.dram_tensor("xs", [b_sh, KT, DS], MDT, kind="ExternalInput")
    xa = nc.dram_tensor("xa", [b_sh, KA, DA], MDT, kind="ExternalInput")
    wpack = nc.dram_tensor("wpack", [b_sh, WPACK], F32, kind="ExternalInput")
    # selector: selg[p, p // KP] = 1
    GB = 64 if b_sh % 64 == 0 else 32  # batch rows per matmul group
    KP = 128 // GB  # k rows folded into the partition dim
    n_groups = b_sh // GB
    selg = nc.dram_tensor("selg", [128, GB], MDT, kind="ExternalInput")
    out = nc.dram_tensor("out", [b_sh, 2], F32, kind="ExternalOutput")

    with tile.TileContext(nc) as tc:
        with (
            tc.tile_pool(name="consts", bufs=1) as consts,
            tc.tile_pool(name="xtp", bufs=6) as xtp,
            tc.tile_pool(name="xsp", bufs=2) as xsp,
            tc.tile_pool(name="xap", bufs=2) as xap,
            tc.tile_pool(name="st2", bufs=1) as st2,
            tc.tile_pool(name="psum", bufs=1, space="PSUM") as psum,
        ):
          for _rep in range(repeat):
            selg_t = consts.tile([128, GB], MDT)
            nc.sync.dma_start(selg_t[:], selg.ap())
            wp = consts.tile([b_sh, WPACK], F32)
            nc.sync.dma_start(wp[:], wpack.ap())

            ps_t = psum.tile([b_sh, DS], F32)
            ps_s = psum.tile([b_sh, DS], F32)
            ps_a = psum.tile([b_sh, DA], F32)

            def reduce_stream(x_ap, K, D, ps_tile, pool):
                """ps_tile[b, :] = sum_k x[b, k, :] via selector matmuls."""
                KR = K // KP  # k rows in the free/chunk dims
                CH = min(KR, 8)  # k rows per SBUF tile
                n_ch = KR // CH
                # PSUM-bank-aligned output slices (bank = 512 fp32)
                dhs = [(lo, min(D, lo + 512)) for lo in range(0, D, 512)]
                for g in range(n_groups):
                    x3 = x_ap[g * GB : (g + 1) * GB].rearrange(
                        "b (k0 kc k1) d -> (b k0) kc (k1 d)", k0=KP, k1=CH
                    )
                    for c in range(n_ch):
                        t = pool.tile([128, CH * D], MDT)
                        nc.sync.dma_start(t[:], x3[:, c, :])
                        for k1 in range(CH):
                            for lo, hi in dhs:
                                nc.tensor.matmul(
                                    ps_tile[g * GB : (g + 1) * GB, lo:hi],
                                    selg_t[:],
                                    t[:, k1 * D + lo : k1 * D + hi],
                                    start=(c == 0 and k1 == 0),
                                    stop=(c == n_ch - 1 and k1 == CH - 1),
                                )

            reduce_stream(xt.ap(), KX, DS, ps_t, xtp)
            reduce_stream(xs.ap(), KT, DS, ps_s, xsp)
            reduce_stream(xa.ap(), KA, DA, ps_a, xap)

            # ---- stage 2: heads (all tiny) ----
            scratch = st2.tile([b_sh, DS], F32)
            s3 = st2.tile([b_sh, 4], F32)
            s3b = st2.tile([b_sh, 4], F32)
            score = st2.tile([b_sh, 4], F32)
            lg = st2.tile([b_sh, 2], F32)
            lgb = st2.tile([b_sh, 2], F32)
            m1 = st2.tile([b_sh, 1], F32)
            nm1 = st2.tile([b_sh, 1], F32)
            ex = st2.tile([b_sh, 2], F32)
            ssum = st2.tile([b_sh, 1], F32)
            inv = st2.tile([b_sh, 1], F32)
            outt = st2.tile([b_sh, 2], F32)

            # s? = dot(p?, wf?)   (accumulate along free dim)
            nc.vector.tensor_tensor_reduce(
                out=scratch[:, 0:DA],
                in0=ps_a[:],
                in1=wp[:, OFF_WFA : OFF_WFA + DA],
                scale=1.0,
                scalar=0.0,
                op0=AL.mult,
                op1=AL.add,
                accum_out=s3[:, 0:1],
            )
            nc.vector.tensor_tensor_reduce(
                out=scratch[:, 0:DS],
                in0=ps_s[:],
                in1=wp[:, OFF_WFT : OFF_WFT + DS],
                scale=1.0,
                scalar=0.0,
                op0=AL.mult,
                op1=AL.add,
                accum_out=s3[:, 1:2],
            )
            nc.vector.tensor_tensor_reduce(
                out=scratch[:, 0:DS],
                in0=ps_t[:],
                in1=wp[:, OFF_WFX : OFF_WFX + DS],
                scale=1.0,
                scalar=0.0,
                op0=AL.mult,
                op1=AL.add,
                accum_out=s3[:, 2:3],
            )
            # + [bfa, bft, bfx]
            nc.vector.tensor_tensor(
                s3b[:, 0:3], s3[:, 0:3], wp[:, OFF_B3 : OFF_B3 + 3], op=AL.add
            )
            nc.scalar.activation(score[:, 0:3], s3b[:, 0:3], ACT.Sigmoid)
            # logits = score @ Wc.T
            nc.vector.tensor_tensor_reduce(
                out=scratch[:, 0:3],
                in0=score[:, 0:3],
                in1=wp[:, OFF_WC0 : OFF_WC0 + 3],
                scale=1.0,
                scalar=0.0,
                op0=AL.mult,
                op1=AL.add,
                accum_out=lg[:, 0:1],
            )
            nc.vector.tensor_tensor_reduce(
                out=scratch[:, 0:3],
                in0=score[:, 0:3],
                in1=wp[:, OFF_WC1 : OFF_WC1 + 3],
                scale=1.0,
                scalar=0.0,
                op0=AL.mult,
                op1=AL.add,
                accum_out=lg[:, 1:2],
            )
            # + bc
            nc.vector.tensor_tensor(
                lgb[:, 0:2], lg[:, 0:2], wp[:, OFF_BC : OFF_BC + 2], op=AL.add
            )
            # softmax over the 2 columns
            nc.vector.tensor_reduce(
                m1[:, 0:1], lgb[:, 0:2], axis=mybir.AxisListType.X, op=AL.max
            )
            nc.vector.tensor_scalar(
                nm1[:, 0:1], m1[:, 0:1], -1.0, None, op0=AL.mult
            )
            nc.scalar.activation(
                ex[:, 0:2], lgb[:, 0:2], ACT.Exp, bias=nm1[:, 0:1], scale=1.0
            )
            nc.vector.tensor_reduce(
                ssum[:, 0:1], ex[:, 0:2], axis=mybir.AxisListType.X, op=AL.add
            )
            nc.vector.reciprocal(inv[:, 0:1], ssum[:, 0:1])
            nc.vector.tensor_scalar(
                outt[:, 0:2], ex[:, 0:2], inv[:, 0:1], None, op0=AL.mult
            )
            nc.sync.dma_start(out.ap(), outt[:, 0:2])

    nc.compile()
    return nc


def make_host_inputs(Wfa, bfa, Wft, bft, Wfx, bfx, Wc, bc, b_sh: int = B_SH,
                     sel_np=np.float32):
    """Build the replicated small-tensor inputs."""
    wpack = np.zeros((WPACK,), np.float32)
    wpack[OFF_WFX : OFF_WFX + DS] = Wfx[0]
    wpack[OFF_WFT : OFF_WFT + DS] = Wft[0]
    wpack[OFF_WFA : OFF_WFA + DA] = Wfa[0]
    wpack[OFF_WC0 : OFF_WC0 + 3] = Wc[0]
    wpack[OFF_WC1 : OFF_WC1 + 3] = Wc[1]
    wpack[OFF_B3 + 0] = bfa[0]
    wpack[OFF_B3 + 1] = bft[0]
    wpack[OFF_B3 + 2] = bfx[0]
    wpack[OFF_BC : OFF_BC + 2] = bc
    wpack_b = np.ascontiguousarray(np.broadcast_to(wpack, (b_sh, WPACK)))

    GB = 64 if b_sh % 64 == 0 else 32
    KP = 128 // GB
    p = np.arange(128)
    selg = np.zeros((128, GB), sel_np)
    selg[p, p // KP] = 1.0
    return wpack_b, selg


_NC_CACHE = {}


def kernel(author_emb, title_emb, text_emb,
           Wa, ba, ca, Wt, bt, ct, Wx, bx, cx,
           Wfa, bfa, Wft, bft, Wfx, bfx, Wc, bc):
    key = "full"
    if key not in _NC_CACHE:
        _NC_CACHE[key] = build_module(B_SH, mm_mode="f16")
    nc = _NC_CACHE[key]

    author_emb = np.asarray(author_emb, np.float32).astype(np.float16)
    title_emb = np.asarray(title_emb, np.float32).astype(np.float16)
    text_emb = np.asarray(text_emb, np.float32).astype(np.float16)
    wpack_b, selg = make_host_inputs(
        np.asarray(Wfa), np.asarray(bfa), np.asarray(Wft), np.asarray(bft),
        np.asarray(Wfx), np.asarray(bfx), np.asarray(Wc), np.asarray(bc),
        sel_np=np.float16,
    )

    in_maps = []
    for c in range(N_CORES):
        sl = slice(c * B_SH, (c + 1) * B_SH)
        in_maps.append(
            {
                "xt": np.ascontiguousarray(text_emb[sl]),
                "xs": np.ascontiguousarray(title_emb[sl]),
                "xa": np.ascontiguousarray(author_emb[sl]),
                "wpack": wpack_b,
                "selg": selg,
            }
        )

    res = run_bass_kernel_spmd(nc, in_maps, core_ids=list(range(N_CORES)))
    return np.concatenate([res.results[c]["out"] for c in range(N_CORES)], axis=0)
